# revision 10
# baseline (speedup 1.0000x reference)
"""Trainium2 Bass kernel for CompressedLinear:
    out = x @ (weight_int8 * scale[:, None]).T + bias

Strategy:
  - Data-parallel over tokens: x [4,2048,4096] -> [8192,4096] -> 8 shards
    of [1024,4096], one per NeuronCore. Weight/scale/bias replicated.
  - Per core: out_c[o, t] = sum_k w[o,k] * x_c[t,k], then *scale[o] + bias[o].
  - All-bf16 matmul: weights are int8-valued (bf16 exact), x is cast to
    bf16 host-side (~0.1% rms rounding, well under the 2e-2 gate).
    bf16 stationary enables Fast Weight Load (f32r's fp32_mode=HIGH
    disables FWL and made each LDWEIGHTS cost 187ns on the PE queue).
  - Weight stationary [128k x 128o] tiles, x moving [128k x 512t] blocks.
  - Output-feature tiles processed in groups of 3 with the k-loop
    interleaved across the group (6 PSUM banks rotating through all 8),
    so the PE has ~40us of work per group and overlaps the initial x load.
  - Warm-up matmuls (N=128 on memset tiles) right after the preamble keep
    the PE HAM clock-gate open (1.2->2.4 GHz) before the first real
    matmul's data lands.
  - Group-0 weights ship in small kt pieces so the first real matmul can
    start as soon as ~380KB have landed; steady-state ships [16kt] blocks.
  - DMA queue split: x on the scalar HW-DGE queue, weights on sync,
    output stores on scalar after x drains.
  - Fused scale+bias on PSUM eviction (DVE tensor_scalar / ACT Identity
    alternating) writing bf16; output [o, t] per core, host-side
    gather/upcast/transpose.
"""

import numpy as np

B, S, IN, OUT = 4, 2048, 4096, 4096
N_CORES = 8
TOK = (B * S) // N_CORES  # 1024 tokens per core
P = 128
KT = IN // P   # 32 k-tiles
OT = OUT // P  # 32 output-feature tiles
NB = 512       # moving free dim per matmul
TB = TOK // NB  # 2 token blocks
# x SBUF chunk sizes in k-tiles: small first chunks so the first matmul
# fires as soon as ~256KB of x has landed, larger ones for DMA efficiency.
# Chunks stay small (<=4 kt) through the group-0 window: a chunk is only
# usable once it has FULLY landed, so big chunks starve the early kt loop.
XCHUNKS = [1, 1, 1, 1, 2, 2, 2, 2, 2, 2, 4, 4, 4, 4]
XSYNC = set()  # the sync queue is saturated with weights until ~35us
WARM_MMS = 32  # dummy N=128 matmuls to hold the PE HAM clock-gate open
# output-feature tiles per interleaved group: a big first group so the
# per-kt x demand during the initial x stream stays under the DMA rate
# (1 kt per 2*G matmuls), then steady groups of 3, and a tiny last group
# so the final eviction tail is short. First group uses all 8 PSUM banks.
GROUP_SIZES = [4, 3, 3, 3, 3, 3, 3, 3, 3, 3, 1]
# w piece sizes in kt units: fine-grained for group 0 (startup latency),
# two half-blocks for everyone else.
WCHUNKS_G0 = [2, 2, 6, 6, 16]
WCHUNKS = [16, 16]

_PROG = None  # (nc, names)


def _build():
    import concourse.mybir as mybir
    import concourse.tile as tile
    from concourse import bacc

    f32 = mybir.dt.float32
    bf16 = mybir.dt.bfloat16

    assert sum(GROUP_SIZES) == OT
    groups = []
    _o = 0
    for g in GROUP_SIZES:
        groups.append(list(range(_o, _o + g)))
        _o += g
    assert sum(XCHUNKS) == KT
    # kt -> (chunk index, offset inside chunk) for x
    kt_map = {}
    _kt = 0
    for ci, sz in enumerate(XCHUNKS):
        for off in range(sz):
            kt_map[_kt] = (ci, off)
            _kt += 1

    def w_piece_map(chunks):
        m = {}
        kt = 0
        for pi, sz in enumerate(chunks):
            for off in range(sz):
                m[kt] = (pi, off)
                kt += 1
        assert kt == KT
        return m

    wmap_g0 = w_piece_map(WCHUNKS_G0)
    wmap = w_piece_map(WCHUNKS)

    nc = bacc.Bacc(None, target_bir_lowering=False, debug=False)
    with tile.TileContext(nc) as tc:
        with tc.tile_pool(name="dram", bufs=1, space="DRAM") as dram:
            xT_d = dram.tile([P, KT, TOK], bf16, kind="ExternalInput", name="xT")
            w_d = dram.tile([OT, P, KT, P], bf16, kind="ExternalInput", name="w")
            sc_d = dram.tile([P, OT], f32, kind="ExternalInput", name="sc")
            bi_d = dram.tile([P, OT], f32, kind="ExternalInput", name="bi")
            out_d = dram.tile([P, OT, TOK], bf16, kind="ExternalOutput", name="out")

            with (
                tc.tile_pool(name="const", bufs=1) as constp,
                tc.tile_pool(name="xp", bufs=1) as xp,
                tc.tile_pool(name="wp", bufs=24) as wp,
                tc.tile_pool(name="op", bufs=2) as outp,
                tc.tile_pool(name="ps", bufs=8, space="PSUM") as psp,
            ):
                sc_sb = constp.tile([P, OT], f32, tag="sc")
                bi_sb = constp.tile([P, OT], f32, tag="bi")

                def w_dma(ot, chunks):
                    # bf16 straight from HBM into the working tiles; one tile
                    # per kt piece so the first matmul doesn't wait for the
                    # whole half-block.
                    tiles = []
                    kt0 = 0
                    for pi, sz in enumerate(chunks):
                        t = wp.tile([P, sz, P], bf16, tag="w", name=f"w{ot}p{pi}")
                        nc.sync.dma_start(t[:], w_d[ot, :, kt0 : kt0 + sz, :])
                        tiles.append(t)
                        kt0 += sz
                    return tiles

                def w_dma_breadth(ots, chunks):
                    # Breadth-first across ots: all ots' piece-0 DMAs first,
                    # then piece-1, ... so the staggered entry (which touches
                    # every ot's low kts early) isn't stuck behind a single
                    # ot's full weight column on the sync queue.
                    tiles = {ot: [] for ot in ots}
                    kt0 = 0
                    for pi, sz in enumerate(chunks):
                        for ot in ots:
                            t = wp.tile(
                                [P, sz, P], bf16, tag="w", name=f"w{ot}p{pi}"
                            )
                            nc.sync.dma_start(t[:], w_d[ot, :, kt0 : kt0 + sz, :])
                            tiles[ot].append(t)
                        kt0 += sz
                    return tiles

                x_tiles = []

                def x_dma(i, eng=None):
                    sz = XCHUNKS[i]
                    k0 = sum(XCHUNKS[:i])
                    t = xp.tile([P, sz, TOK], bf16, tag=f"x{i}", name=f"x{i}")
                    (eng or nc.scalar).dma_start(t[:], xT_d[:, k0 : k0 + sz, :])
                    x_tiles.append(t)

                # Startup order: x chunks stream on the scalar queue from t=0;
                # weights stream on the sync queue concurrently, first pieces
                # small. scale/bias aren't needed until the first eviction.
                x_dma(0)
                w_tiles = {}
                g0_tiles = w_dma_breadth(groups[0], WCHUNKS_G0)
                for ot in groups[0]:
                    w_tiles[ot] = (g0_tiles[ot], wmap_g0)
                x_dma(1)
                nc.scalar.dma_start(sc_sb[:], sc_d[:])
                nc.scalar.dma_start(bi_sb[:], bi_d[:])
                for i in range(2, len(XCHUNKS)):
                    x_dma(i, eng=nc.sync if i in XSYNC else None)

                if WARM_MMS:
                    # Warm-up: dummy bf16 matmuls on memset tiles keep the PE
                    # busy so the HAM clock-gate opens (1.2->2.4 GHz) before
                    # the first real matmul's data lands.
                    wu_w = constp.tile([P, P], bf16, tag="wu_w")
                    wu_x = constp.tile([P, P], bf16, tag="wu_x")
                    nc.vector.memset(wu_w[:], 0.0)
                    nc.vector.memset(wu_x[:], 0.0)
                    wu_ps = [
                        psp.tile([P, NB], f32, tag="ps", name=f"wu_ps{i}")
                        for i in range(2)
                    ]
                    for i in range(WARM_MMS):
                        nc.tensor.matmul(
                            wu_ps[i % 2][:, 0:P], wu_w[:], wu_x[:],
                            start=True, stop=True,
                        )

                for gi, group in enumerate(groups):
                    # Prefetch next group's weights.
                    if gi + 1 < len(groups):
                        for ot in groups[gi + 1]:
                            w_tiles[ot] = (w_dma(ot, WCHUNKS), wmap)
                    ps = {}
                    for i, ot in enumerate(group):
                        for tb in range(TB):
                            ps[(ot, tb)] = psp.tile(
                                [P, NB], f32, tag="ps", name=f"ps{ot}_{tb}"
                            )

                    def mm(ot, kt, tbs=tuple(range(TB))):
                        ci, off = kt_map[kt]
                        xt = x_tiles[ci]
                        wts, wm = w_tiles[ot]
                        pi, woff = wm[kt]
                        wt = wts[pi]
                        for tb in tbs:
                            nc.tensor.matmul(
                                ps[(ot, tb)][:],
                                wt[:, woff, :],
                                xt[:, off, tb * NB : (tb + 1) * NB],
                                start=(kt == 0),
                                stop=(kt == KT - 1),
                            )

                    # One staging tile per group: the group's ots are
                    # adjacent in out_d, so a single store writes G*2KB
                    # contiguous per partition. Per-(ot,tb) stores were 1KB
                    # lines draining at ~18GB/s; evictions then stalled on
                    # the staging-pool recycle and held PSUM banks hostage.
                    G = len(group)
                    o_g = outp.tile([P, G, TOK], bf16, tag="o", name=f"o_g{gi}")

                    def evict(ot, tbs=tuple(range(TB))):
                        i = ot - group[0]
                        for tb in tbs:
                            dst = o_g[:, i, tb * NB : (tb + 1) * NB]
                            if tb % 2 == 0:
                                nc.vector.tensor_scalar(
                                    dst,
                                    ps[(ot, tb)][:],
                                    sc_sb[:, ot : ot + 1],
                                    bi_sb[:, ot : ot + 1],
                                    op0=mybir.AluOpType.mult,
                                    op1=mybir.AluOpType.add,
                                )
                            else:
                                # out = Identity(in*scale + bias) on ScalarE;
                                # splits eviction across two engines.
                                nc.scalar.activation(
                                    dst,
                                    ps[(ot, tb)][:],
                                    mybir.ActivationFunctionType.Identity,
                                    bias=bi_sb[:, ot : ot + 1],
                                    scale=sc_sb[:, ot : ot + 1],
                                )

                    if gi == 0:
                        # Group 0 runs kt-major from kt0: x arrives
                        # kt-serially, so consuming each kt across all 4 ots
                        # (8 matmuls/kt) keeps PE demand under the DMA rate.
                        kt_start = 0
                    else:
                        # Staggered entry: each ot runs kt 0-3 alone, in the
                        # same order the previous group's ots were evicted,
                        # so PSUM banks hand over progressively.
                        for ot in group:
                            for kt in range(4):
                                mm(ot, kt)
                        kt_start = 4
                    # Interleaved k-loop over all but the last 4 kts, then a
                    # staggered finish: each ot runs kt 28-31 back-to-back and
                    # is evicted immediately, so PSUM banks free progressively
                    # and the next group's matmuls never wait on eviction.
                    for kt in range(kt_start, KT - 4):
                        for ot in group:
                            mm(ot, kt)
                    last_g = gi == len(groups) - 1
                    for oi, ot in enumerate(group):
                        if last_g and oi == len(group) - 1:
                            # Final ot: finish tb0 first so its eviction
                            # overlaps tb1's last matmuls, shortening the
                            # tail chain after the very last matmul.
                            for kt in range(KT - 4, KT):
                                mm(ot, kt, tbs=(0,))
                            evict(ot, tbs=(0,))
                            for kt in range(KT - 4, KT):
                                mm(ot, kt, tbs=(1,))
                            evict(ot, tbs=(1,))
                        else:
                            for kt in range(KT - 4, KT):
                                mm(ot, kt)
                            evict(ot)
                    # sync HW-DGE queue: stays hot with weight prefetches, so
                    # store descriptors drain promptly (the scalar queue goes
                    # quiet after x finishes and drained stores at ~18GB/s)
                    nc.sync.dma_start(
                        out_d[:, group[0] : group[0] + G, :], o_g[:]
                    )
    nc.compile()
    names = {
        "xT": xT_d.tensor.name,
        "w": w_d.tensor.name,
        "sc": sc_d.tensor.name,
        "bi": bi_d.tensor.name,
        "out": out_d.tensor.name,
    }
    return nc, names


def _get_prog():
    global _PROG
    if _PROG is None:
        _PROG = _build()
    return _PROG


def _marshal(x, weight_int8, scale, bias):
    import ml_dtypes

    # weight [o, k] -> [ot, p(k), kt, ol]; bf16 is exact for int8 values
    w = np.asarray(weight_int8, dtype=np.float32).astype(ml_dtypes.bfloat16)
    w_m = np.ascontiguousarray(
        w.reshape(OT, P, KT, P).transpose(0, 3, 2, 1)
    )
    sc_m = np.ascontiguousarray(np.asarray(scale, np.float32).reshape(OT, P).T)
    bi_m = np.ascontiguousarray(np.asarray(bias, np.float32).reshape(OT, P).T)
    x_flat = np.asarray(x, np.float32).reshape(B * S, IN).astype(ml_dtypes.bfloat16)
    x_shards = []
    for c in range(N_CORES):
        sh = x_flat[c * TOK : (c + 1) * TOK]  # [t, k]
        x_shards.append(
            np.ascontiguousarray(sh.reshape(TOK, KT, P).transpose(2, 1, 0))
        )
    return w_m, sc_m, bi_m, x_shards


def _run(x, weight_int8, scale, bias, trace=False):
    from concourse.bass_utils import run_bass_kernel_spmd

    nc, names = _get_prog()
    w_m, sc_m, bi_m, x_shards = _marshal(x, weight_int8, scale, bias)
    in_maps = [
        {
            names["xT"]: x_shards[c],
            names["w"]: w_m,
            names["sc"]: sc_m,
            names["bi"]: bi_m,
        }
        for c in range(N_CORES)
    ]
    res = run_bass_kernel_spmd(
        nc, in_maps, core_ids=list(range(N_CORES)), trace=trace
    )
    full = np.empty((B * S, OUT), dtype=np.float32)
    for c in range(N_CORES):
        out_c = np.asarray(res.results[c][names["out"]], dtype=np.float32)  # [p, ot, t]
        full[c * TOK : (c + 1) * TOK] = out_c.transpose(2, 1, 0).reshape(TOK, OUT)
    return full.reshape(B, S, OUT), res


def kernel(x, weight_int8, scale, bias):
    out, _ = _run(x, weight_int8, scale, bias, trace=False)
    return out


def kernel_traced(x, weight_int8, scale, bias):
    out, res = _run(x, weight_int8, scale, bias, trace=True)
    return out, res
